# revision 34
# baseline (speedup 1.0000x reference)
"""Trainium2 Bass kernel for nn_Decoder (LSTM-cell decoder + dot-product
attention + tied-embedding projection), data-parallel over batch on 8 cores.

Contract: kernel(**inputs) takes the FULL unsharded numpy inputs (as produced
by the reference setup_inputs) and returns the FULL (B, VOCAB, T) fp32 output.

Strategy per core (4 batches each):
  - host: shift y, gather embeddings, transpose to (EMBED, ntok) layout
  - device: LSTM1/LSTM2 (zero-state cells; f-gate skipped), dot-product
    attention with length masking (mask added via K=1 ones-matmul into the
    PSUM accumulation group), softmax on free dim, PE-transpose of attn,
    context matmul, then the dominant (EMBED x VOCAB) tied projection with
    E^T streamed from HBM as fp16.
  - logits are written to DRAM as fp16 (halves the dominant 131MB/core
    output write; adds ~3e-4 RMS rel err) and upconverted on host.
  - DMAs are batched (4 vocab tiles per store, 1 load per E chunk, merged
    input loads) to keep descriptor-generation off the critical path, and
    PSUM->SBUF logit copies rotate across Act/DVE/Pool engines.
"""
import sys, os

for _p in ("/opt/trn_rl_repo",):
    if _p not in sys.path and os.path.isdir(_p):
        sys.path.insert(0, _p)

import numpy as np

VOCAB = 32000
EMBED = 256
H1 = 512
KV = 128
B = 32
S = 512
T = 256
N_CORES = 8
BPC = B // N_CORES          # 4 batches per core
NTOK = BPC * T              # 1024 tokens per core
NVT = VOCAB // 128          # 250 vocab tiles
VCHUNK = 16                 # vocab tiles per E-stream load
NCHUNK = (NVT + VCHUNK - 1) // VCHUNK  # 16 (last chunk has 10 tiles)
SG = 8                      # vocab tiles per batched output store group

_COMPILED = None


def _build_module():
    import concourse.tile as tile
    import concourse.mybir as mybir
    from concourse import bacc

    F32 = mybir.dt.float32
    F32R = mybir.dt.float32r
    F16 = mybir.dt.float16
    AF = mybir.ActivationFunctionType
    AX = mybir.AxisListType
    ALU = mybir.AluOpType

    nc = bacc.Bacc("TRN2", target_bir_lowering=False, debug=False,
                   num_devices=N_CORES)

    def din(name, shape, dt=F32):
        return nc.dram_tensor(name, list(shape), dt, kind="ExternalInput").ap()

    xT_d = din("xT", (EMBED, NTOK), F16)       # embedded tokens, transposed
    keyT_d = din("keyT", (BPC, KV, S), F16)    # key[b].T
    val_d = din("val", (BPC, S, KV), F16)      # value[b]
    w1T_d = din("w1T", (EMBED, 3 * H1), F16)   # W_ih1[i,g,o].T
    w2T_d = din("w2T", (H1, 3 * KV), F16)      # W_ih2[i,g,o].T
    # misc: cols [0:12]=b1 tiled, [12:15]=b2 tiled, [15:265]=b_out tiled
    misc_d = din("misc", (128, 265), F32)
    ident_d = din("ident", (128, 128), F32R)
    # row: [0:BPC*S]=additive mask (0/-30000), [BPC*S:+128]=ones; fp16 so
    # the K=1 mask-add matmul matches the fp16 energy matmul's dtype.
    row_d = din("rowv", (1, BPC * S + 128), F16)
    ET_d = din("ET", (EMBED, VOCAB), F16)      # E.T in fp16
    out_d = nc.dram_tensor("out", [BPC, VOCAB, T], F16,
                           kind="ExternalOutput").ap()
    out_r = out_d.rearrange("b v t -> v b t")  # (VOCAB, BPC, T) view

    with tile.TileContext(nc) as tc:
        import contextlib
        ctx = contextlib.ExitStack()
        with ctx:
            sb = ctx.enter_context(tc.tile_pool(name="sb", bufs=1))
            work = ctx.enter_context(tc.tile_pool(name="work", bufs=4))
            epool = ctx.enter_context(tc.tile_pool(name="epool", bufs=10))
            opool = ctx.enter_context(tc.tile_pool(name="opool", bufs=6))
            ps512 = ctx.enter_context(
                tc.tile_pool(name="ps512", bufs=6, space="PSUM"))
            pstr = ctx.enter_context(
                tc.tile_pool(name="pstr", bufs=2, space="PSUM"))

            # ------- resident inputs (7 batched DMAs, scalar queue) ------
            # All on the scalar HWDGE queue, ahead of the E-chunk prefetch
            # loads (same queue => strict FIFO), so the E stream cannot
            # starve the critical path; stores get the SP queue to
            # themselves. Bias/weight loads first so LSTM1 starts ASAP.
            xTt = sb.tile([128, 2 * NTOK], F16, name="xTt")
            nc.scalar.dma_start(
                xTt[:].rearrange("p (k t) -> p k t", k=2),
                xT_d.rearrange("(k p) t -> p k t", p=128))
            w1t = sb.tile([128, 2 * 3 * H1], F16, name="w1t")
            nc.scalar.dma_start(
                w1t[:].rearrange("p (k c) -> p k c", k=2),
                w1T_d.rearrange("(k p) c -> p k c", p=128))
            misc = sb.tile([128, 265], F32)
            nc.scalar.dma_start(misc[:], misc_d[:])
            ident = sb.tile([128, 128], F32R)
            nc.scalar.dma_start(ident[:], ident_d[:])
            w2t = sb.tile([128, 4 * 3 * KV], F16, name="w2t")
            nc.scalar.dma_start(
                w2t[:].rearrange("p (k c) -> p k c", k=4),
                w2T_d.rearrange("(k p) c -> p k c", p=128))
            rowt = sb.tile([1, BPC * S + 128], F16)
            nc.scalar.dma_start(rowt[:], row_d[:])
            keyT = sb.tile([128, BPC * S], F16)
            nc.scalar.dma_start(
                keyT[:].rearrange("p (b s) -> p b s", b=BPC),
                keyT_d.rearrange("b p s -> p b s"))
            valt = sb.tile([128, BPC * S], F16)  # value k-tiles side by side
            nc.scalar.dma_start(
                valt[:].rearrange("p (b st v) -> p b st v", b=BPC, st=4),
                val_d.rearrange("b (st p) v -> p b st v", p=128))

            b1 = misc[:, 0:12]
            b2 = misc[:, 12:15]
            bout = misc[:, 15:265]
            maskb = rowt[0:1, 0:BPC * S]
            ones1 = rowt[0:1, BPC * S:BPC * S + 128]

            h1T = [sb.tile([128, NTOK], F16, name=f"h1T{k}") for k in range(4)]
            ctxT = sb.tile([128, NTOK], F16)
            h2L = sb.tile([128, NTOK], F16, name="h2L")

            def r(ap):
                return ap if ap.dtype == F32R else ap.bitcast(F32R)

            # ---------------- phase helpers ----------------
            # gates^T tiles: m = 0..3 -> i, 4..7 -> g, 8..11 -> o
            def lstm1(n):
                tok = slice(n * 512, (n + 1) * 512)
                for msub in range(4):
                    pg = {}
                    for gi, gname in enumerate(("i", "g", "o")):
                        m = gi * 4 + msub
                        ps = ps512.tile([128, 512], F32, name="ps_mm",
                                        tag="ps_mm")
                        for k in range(2):
                            nc.tensor.matmul(
                                ps[:],
                                w1t[:, k * 1536 + m * 128:
                                    k * 1536 + (m + 1) * 128],
                                xTt[:, k * NTOK + n * 512:
                                    k * NTOK + (n + 1) * 512],
                                start=(k == 0), stop=(k == 1))
                        pg[gname] = (ps, m)
                    # sig_o issued before tanh_c so ACT isn't stalled on the
                    # DVE product while it still has independent work.
                    sig_i = work.tile([128, 512], F32, tag="lstm_act")
                    nc.scalar.activation(sig_i[:], pg["i"][0][:], AF.Sigmoid,
                                         bias=b1[:, pg["i"][1]:pg["i"][1] + 1])
                    tanh_g = work.tile([128, 512], F32, tag="lstm_act")
                    nc.scalar.activation(tanh_g[:], pg["g"][0][:], AF.Tanh,
                                         bias=b1[:, pg["g"][1]:pg["g"][1] + 1])
                    sig_o = work.tile([128, 512], F32, tag="lstm_act")
                    nc.scalar.activation(sig_o[:], pg["o"][0][:], AF.Sigmoid,
                                         bias=b1[:, pg["o"][1]:pg["o"][1] + 1])
                    cst = work.tile([128, 512], F32, tag="lstm_act")
                    nc.vector.tensor_mul(cst[:], sig_i[:], tanh_g[:])
                    tanh_c = work.tile([128, 512], F32, tag="lstm_act")
                    nc.scalar.activation(tanh_c[:], cst[:], AF.Tanh)
                    nc.vector.tensor_mul(h1T[msub][:, tok], sig_o[:],
                                         tanh_c[:])

            def lstm2(n):
                tok = slice(n * 512, (n + 1) * 512)
                pg = {}
                for gi, gname in enumerate(("i", "g", "o")):
                    ps = ps512.tile([128, 512], F32, name="ps_mm", tag="ps_mm")
                    for k in range(4):
                        nc.tensor.matmul(
                            ps[:],
                            w2t[:, k * 384 + gi * 128:
                                k * 384 + (gi + 1) * 128],
                            h1T[k][:, tok],
                            start=(k == 0), stop=(k == 3))
                    pg[gname] = ps
                sig_i = work.tile([128, 512], F32, tag="lstm_act")
                nc.scalar.activation(sig_i[:], pg["i"][:], AF.Sigmoid,
                                     bias=b2[:, 0:1])
                tanh_g = work.tile([128, 512], F32, tag="lstm_act")
                nc.scalar.activation(tanh_g[:], pg["g"][:], AF.Tanh,
                                     bias=b2[:, 1:2])
                sig_o = work.tile([128, 512], F32, tag="lstm_act")
                nc.scalar.activation(sig_o[:], pg["o"][:], AF.Sigmoid,
                                     bias=b2[:, 2:3])
                cst = work.tile([128, 512], F32, tag="lstm_act")
                nc.vector.tensor_mul(cst[:], sig_i[:], tanh_g[:])
                tanh_c = work.tile([128, 512], F32, tag="lstm_act")
                nc.scalar.activation(tanh_c[:], cst[:], AF.Tanh)
                nc.vector.tensor_mul(h2L[:, tok], sig_o[:], tanh_c[:])

            def attn_batch(b):
                attnT = [work.tile([128, T], F16, tag=f"attnT{st}",
                                   name=f"attnT{st}")
                         for st in range(4)]
                for tt in range(2):
                    tcol = b * T + tt * 128
                    ps_e = ps512.tile([128, 512], F32, name="ps_mm",
                                      tag="ps_mm")
                    nc.tensor.matmul(ps_e[:],
                                     h2L[:, tcol:tcol + 128],
                                     keyT[:, b * S:(b + 1) * S],
                                     start=True, stop=False)
                    nc.tensor.matmul(ps_e[:], ones1,
                                     maskb[0:1, b * S:(b + 1) * S],
                                     start=False, stop=True)
                    # no max-subtraction: energies are O(30), exp stays well
                    # inside fp32 range; masked lanes underflow to exactly 0.
                    attn = work.tile([128, 512], F32, tag="attn")
                    rowsum = work.tile([128, 1], F32, tag="stat")
                    nc.scalar.activation(attn[:], ps_e[:], AF.Exp,
                                         accum_out=rowsum[:, 0:1])
                    recip = work.tile([128, 1], F32, tag="stat")
                    nc.vector.reciprocal(recip[:], rowsum[:])
                    attn_n = work.tile([128, 512], F32R, tag="attn_n")
                    nc.vector.tensor_scalar_mul(attn_n[:], attn[:],
                                                recip[:, 0:1])
                    for st in range(4):
                        ps_t = pstr.tile([128, 128], F32R, name="ps_tr",
                                         tag="ps_tr")
                        nc.tensor.transpose(
                            ps_t[:],
                            r(attn_n[:, st * 128:(st + 1) * 128]),
                            ident[:])
                        dst = attnT[st][:, tt * 128:(tt + 1) * 128]
                        if st % 2 == 0:
                            nc.scalar.copy(dst, ps_t[:])
                        else:
                            nc.vector.tensor_copy(dst, ps_t[:])
                ps_c = ps512.tile([128, 512], F32, name="ps_mm", tag="ps_mm")
                for st in range(4):
                    nc.tensor.matmul(
                        ps_c[:, 0:T],
                        valt[:, (b * 4 + st) * 128:(b * 4 + st + 1) * 128],
                        attnT[st][:],
                        start=(st == 0), stop=(st == 3))
                if b % 2 == 0:
                    nc.scalar.copy(ctxT[:, b * T:(b + 1) * T], ps_c[:, 0:T])
                else:
                    nc.vector.tensor_copy(ctxT[:, b * T:(b + 1) * T],
                                          ps_c[:, 0:T])

            # ------------- logits: out = [h2; ctx]^T . [E_lo; E_hi] -------
            # E streamed one chunk (VCHUNK vocab tiles, both K halves) per
            # DMA; per-(chunk, token-half) store groups so stores can begin
            # before the second half of the decoder head has run; PSUM->SBUF
            # copies (b_out bias add + fp16 cast) rotate Act/DVE/Pool.
            cp = [0]

            def copy_bias(dst, src, bias_ap):
                # GPSIMD/Pool cannot read PSUM on TRN2 hardware, so the
                # PSUM->SBUF bias-add copies alternate Act/DVE only.
                k = cp[0] % 2
                cp[0] += 1
                if k == 0:
                    nc.scalar.activation(dst, src, AF.Identity, bias=bias_ap)
                else:
                    nc.vector.tensor_scalar_add(dst, src, bias_ap)

            et_tiles = {}

            def et_of(ci):
                if ci not in et_tiles:
                    nv = min(VCHUNK, NVT - ci * VCHUNK)
                    ecols = nv * 128
                    base = ci * VCHUNK * 128
                    et = epool.tile([128, 2 * VCHUNK * 128], F16, tag="et")
                    nc.scalar.dma_start(
                        et[:, :2 * ecols].rearrange("p (k c) -> p k c", k=2),
                        ET_d.rearrange("(k p) v -> p k v", p=128)[
                            :, :, base:base + ecols])
                    et_tiles[ci] = et
                return et_tiles[ci]

            def logits_half(ci, half):
                nv = min(VCHUNK, NVT - ci * VCHUNK)
                ecols = nv * 128
                et = et_of(ci)
                tok = slice(half * 512, (half + 1) * 512)
                for sub in range(0, nv, SG):
                    ns = min(SG, nv - sub)
                    v0 = ci * VCHUNK + sub
                    osb = opool.tile([128, SG * 512], F16, tag="osb")
                    for j2 in range(ns):
                        j = sub + j2
                        v = v0 + j2
                        ps_l = ps512.tile([128, 512], F32, name="ps_mm",
                                          tag="ps_mm")
                        nc.tensor.matmul(ps_l[:],
                                         et[:, j * 128:(j + 1) * 128],
                                         h2L[:, tok],
                                         start=True, stop=False)
                        nc.tensor.matmul(ps_l[:],
                                         et[:, ecols + j * 128:
                                            ecols + (j + 1) * 128],
                                         ctxT[:, tok],
                                         start=False, stop=True)
                        copy_bias(osb[:, j2 * 512:(j2 + 1) * 512],
                                  ps_l[:], bout[:, v:v + 1])
                    # store this half's two batches: (p, ns vtiles, T)
                    for b2 in range(2):
                        b = 2 * half + b2
                        nc.sync.dma_start(
                            out_d[b, v0 * 128:(v0 + ns) * 128, :]
                            .rearrange("(j p) t -> p j t", p=128),
                            osb[:, :ns * 512].rearrange(
                                "p (j b t) -> p b j t",
                                j=ns, b=2)[:, b2])

            # ------------- interleaved schedule ---------------------------
            # Token half 0 (batches 0,1) runs through the full head first,
            # then logits-half-0 of the first chunks starts streaming stores
            # while half 1 of the head executes. The short (10-tile) E chunk
            # goes first so the kernel tail is a full, well-pipelined chunk.
            # Half-0 logits of 7 chunks interleave with the second head pass
            # so neither PE nor the DMA store stream ever goes idle; their
            # E tiles (7) stay held until the half-1 pass drains them.
            EARLY = [NCHUNK - 1, 0, 1, 2, 3, 4, 5]
            lstm1(0)
            lstm2(0)
            attn_batch(0)
            attn_batch(1)
            for ci in EARLY[:4]:
                logits_half(ci, 0)
            lstm1(1)
            logits_half(EARLY[4], 0)
            lstm2(1)
            logits_half(EARLY[5], 0)
            attn_batch(2)
            logits_half(EARLY[6], 0)
            attn_batch(3)
            for ci in EARLY:
                logits_half(ci, 1)
            for ci in range(6, NCHUNK - 1):
                logits_half(ci, 0)
                logits_half(ci, 1)

    nc.compile()
    return nc


def _prep_inputs(key, value, encoder_len, y, E, W_ih1, b_ih1, b_hh1,
                 W_ih2, b_ih2, b_hh2, b_out):
    """Host-side prep: shard over batch, gather embeddings, build transposed
    weight/bias layouts shared by all cores."""
    key = np.asarray(key, dtype=np.float32)
    value = np.asarray(value, dtype=np.float32)
    encoder_len = np.asarray(encoder_len)
    y = np.asarray(y)
    E = np.asarray(E, dtype=np.float32)

    # shifted inputs + embedding gather (host): (B, T) -> (B, T, EMBED)
    inputs = np.concatenate(
        [np.zeros((B, 1), dtype=y.dtype), y[:, :-1]], axis=1)
    embed = E[inputs]                                  # (B, T, EMBED)

    # LSTM weights, f-gate dropped (zero-state cell never uses it)
    def gate_sel(W, H):
        return np.concatenate([W[0:H], W[2 * H:3 * H], W[3 * H:4 * H]], axis=0)

    w1 = gate_sel(np.asarray(W_ih1, np.float32), H1)       # (1536, 256)
    w1T = np.ascontiguousarray(w1.T).astype(np.float16)    # (256, 1536)
    bb1 = gate_sel((np.asarray(b_ih1, np.float32)
                    + np.asarray(b_hh1, np.float32))[:, None], H1)[:, 0]
    b1t = np.ascontiguousarray(bb1.reshape(12, 128).T)     # (128, 12)
    w2 = gate_sel(np.asarray(W_ih2, np.float32), KV)       # (384, 512)
    w2T = np.ascontiguousarray(w2.T).astype(np.float16)    # (512, 384)
    bb2 = gate_sel((np.asarray(b_ih2, np.float32)
                    + np.asarray(b_hh2, np.float32))[:, None], KV)[:, 0]
    b2t = np.ascontiguousarray(bb2.reshape(3, 128).T)      # (128, 3)
    ET = np.ascontiguousarray(E.T).astype(np.float16)      # (256, 32000)
    boutt = np.ascontiguousarray(
        np.asarray(b_out, np.float32).reshape(NVT, 128).T)  # (128, 250)

    misc = np.concatenate([b1t, b2t, boutt], axis=1)       # (128, 265)

    smask = (np.arange(S)[None, :] >= np.asarray(encoder_len)[:, None])
    # -30000 is fp16-representable; exp(-30000 + O(30)) underflows to 0.
    maskb = np.where(smask, np.float32(-30000.0), np.float32(0.0))  # (B, S)

    in_maps = []
    for c in range(N_CORES):
        bs = slice(c * BPC, (c + 1) * BPC)
        xT = np.ascontiguousarray(
            embed[bs].reshape(NTOK, EMBED).T).astype(np.float16)  # (256, 1024)
        keyT = np.ascontiguousarray(
            key[bs].transpose(0, 2, 1)).astype(np.float16)  # (4, 128, 512)
        rowv = np.concatenate(
            [maskb[bs].reshape(1, BPC * S),
             np.ones((1, 128), np.float32)],
            axis=1).astype(np.float16)                     # (1, 2176)
        in_maps.append({
            "xT": xT,
            "keyT": keyT,
            "val": np.ascontiguousarray(value[bs]).astype(np.float16),
            "w1T": w1T,
            "w2T": w2T,
            "misc": misc,
            "ident": np.eye(128, dtype=np.float32),
            "rowv": np.ascontiguousarray(rowv),
            "ET": ET,
        })
    return in_maps


def _get_compiled():
    global _COMPILED
    if _COMPILED is None:
        _COMPILED = _build_module()
    return _COMPILED


def kernel(key, value, encoder_len, y, E, W_ih1, b_ih1, b_hh1,
           W_ih2, b_ih2, b_hh2, b_out):
    from concourse.bass_utils import run_bass_kernel_spmd

    nc = _get_compiled()
    in_maps = _prep_inputs(key, value, encoder_len, y, E, W_ih1, b_ih1, b_hh1,
                           W_ih2, b_ih2, b_hh2, b_out)
    res = run_bass_kernel_spmd(nc, in_maps, core_ids=list(range(N_CORES)))
    out = np.concatenate([res.results[c]["out"] for c in range(N_CORES)],
                         axis=0)
    return np.ascontiguousarray(out, dtype=np.float32)
